# revision 3
# baseline (speedup 1.0000x reference)
"""Two-layer SAGEConv GNN on 8 Trainium2 NeuronCores.

Strategy (graph/data parallel per sharding hint):
  - Nodes sharded across 8 cores (8750 rows each, padded to 9216).
  - Layer projections via bf16 matmuls on TensorE (per-core row shards,
    weights replicated; bias folded into an augmented constant-1 input row).
  - Mean aggregation: edges bucketed by destination window (128 dst rows),
    sorted by src; per 128-edge block, gather h[src] rows via indirect DMA
    and accumulate St_block^T @ Msg_block into the window's PSUM tile, where
    St is the per-block one-hot (edge -> dst-within-window) matrix built on
    host. Window PSUM is then scaled by 1/deg, combined with the self path
    and written out.
  - The halo/all-gather of projections between layers happens at the launch
    boundary: each launch returns per-core shards; the host concatenates and
    feeds the full (replicated) projection table to the next launch.

Three SPMD launches: L1 (input projections), L2 (layer-1 aggregation +
layer-2 projections), L3 (layer-2 aggregation + output).
"""
import os
import sys
import types
import contextlib
import ctypes

import numpy as np
import ml_dtypes

import concourse.bass as bass
import concourse.bacc as bacc
import concourse.mybir as mybir
import concourse.tile as tile
from concourse import bass_utils

# ---------------------------------------------------------------- constants
N_NODES = 70000
N_EDGES = 500000
C_IN, C_HID, C_OUT = 1044, 128, 64
NCORES = 8
P = 128
SHARD = N_NODES // NCORES            # 8750
R = 9216                             # padded rows per core (multiple of 512)
NWIN = R // P                        # 72 windows per core
CIN_PAD = 1152                       # 9 * 128 (row 1044 is the bias row)
CT = CIN_PAD // P                    # 9 contraction tiles
RSUP = 512                           # row super-block for X loads
BF16 = mybir.dt.bfloat16
F32 = mybir.dt.float32
I32 = mybir.dt.int32

_EXEC_NS = []                        # exec_time_ns per launch when profiling


# ------------------------------------------------------------- host helpers
def _bf16(x):
    return np.asarray(x, np.float32).astype(ml_dtypes.bfloat16)


def _prep_edges(src, dst):
    """Per-core edge slot layout.

    Returns (B, offs[NCORES][P,B] int32, st[NCORES][P,B*P] bf16,
             invdeg[NCORES][P,NWIN] f32).
    Slots: per core, edges grouped by dst window (dst_local // 128), sorted
    by src inside a window, padded per window to k_w*128 slots where k_w is
    the max block count over cores (>=1).  Padding slots gather row 0 and
    have an all-zero St row.
    """
    deg = np.bincount(dst, minlength=N_NODES).astype(np.float32)
    inv_deg = 1.0 / np.maximum(deg, 1.0)

    core = dst // SHARD
    dst_local = dst - core * SHARD
    win = dst_local // P
    dstrel = dst_local - win * P
    # remap src into padded global row space
    src_pad = (src // SHARD) * R + (src % SHARD)

    per_core = []
    counts = np.zeros((NCORES, NWIN), np.int64)
    for m in range(NCORES):
        sel = np.nonzero(core == m)[0]
        order = np.lexsort((src[sel], win[sel]))
        sel = sel[order]
        w_sorted = win[sel]
        cnt = np.bincount(w_sorted, minlength=NWIN)
        counts[m] = cnt
        per_core.append((sel, w_sorted, cnt))

    k_w = np.maximum(1, (counts.max(axis=0) + P - 1) // P)   # blocks per window
    B = int(k_w.sum())

    offs_all, st_all, invd_all = [], [], []
    for m in range(NCORES):
        sel, w_sorted, cnt = per_core[m]
        offs = np.zeros((B * P,), np.int32)
        rel = np.full((B * P,), -1, np.int64)
        bstart = np.concatenate(([0], np.cumsum(k_w)))  # block idx per window
        pos = 0
        for w in range(NWIN):
            n = cnt[w]
            s = bstart[w] * P
            e_idx = sel[pos:pos + n]
            offs[s:s + n] = src_pad[e_idx]
            rel[s:s + n] = dstrel[e_idx]
            pos += n
        # one-hot St: [slot, dstrel] -> layout [P, B, P] with slot%P on axis0
        st = np.zeros((B * P, P), ml_dtypes.bfloat16)
        valid = rel >= 0
        st[np.nonzero(valid)[0], rel[valid]] = 1.0
        st = st.reshape(B, P, P).transpose(1, 0, 2).reshape(P, B * P)
        st = np.ascontiguousarray(st)
        offs = np.ascontiguousarray(offs.reshape(B, P).T)

        invd = inv_deg[m * SHARD:(m + 1) * SHARD]
        invd = np.concatenate([invd, np.ones((R - SHARD,), np.float32)])
        invd = np.ascontiguousarray(invd.reshape(NWIN, P).T)

        offs_all.append(offs)
        st_all.append(st)
        invd_all.append(invd)
    return B, offs_all, st_all, invd_all


# ------------------------------------------------------------ device builds
def _build_l1():
    nc = bacc.Bacc("TRN2", target_bir_lowering=False, debug=False,
                   num_devices=NCORES)
    xt = nc.dram_tensor("xt", [CIN_PAD, R], F32, kind="ExternalInput")
    w1 = nc.dram_tensor("w1", [CIN_PAD, 2 * C_HID], BF16, kind="ExternalInput")
    h_out = nc.dram_tensor("h_out", [R, C_HID], BF16, kind="ExternalOutput")
    xr_out = nc.dram_tensor("xr_out", [R, C_HID], F32, kind="ExternalOutput")

    with tile.TileContext(nc) as tc:
        with tc.tile_pool(name="wp", bufs=1) as wp, \
             tc.tile_pool(name="xp", bufs=2) as xp, \
             tc.tile_pool(name="ev", bufs=3) as ev, \
             tc.tile_pool(name="ps", bufs=2, space="PSUM") as ps:
            w1t = wp.tile([P, CT * 2 * C_HID], BF16)
            for t in range(CT):
                nc.sync.dma_start(
                    out=w1t[:, t * 2 * C_HID:(t + 1) * 2 * C_HID],
                    in_=w1[t * P:(t + 1) * P, :])
            for rs in range(R // RSUP):
                xtile = xp.tile([P, CT * RSUP], BF16, tag="xtile")
                for t in range(CT):
                    nc.gpsimd.dma_start(
                        out=xtile[:, t * RSUP:(t + 1) * RSUP],
                        in_=xt[t * P:(t + 1) * P, rs * RSUP:(rs + 1) * RSUP])
                for j in range(RSUP // P):
                    acc = ps.tile([P, 2 * C_HID], F32, space="PSUM", tag="acc")
                    for t in range(CT):
                        nc.tensor.matmul(
                            out=acc[:],
                            lhsT=xtile[:, (t * RSUP + j * P):(t * RSUP + j * P + P)],
                            rhs=w1t[:, t * 2 * C_HID:(t + 1) * 2 * C_HID],
                            start=(t == 0), stop=(t == CT - 1))
                    r0 = rs * RSUP + j * P
                    hst = ev.tile([P, C_HID], BF16, tag="hst")
                    nc.scalar.copy(out=hst[:], in_=acc[:, :C_HID])
                    nc.sync.dma_start(out=h_out[r0:r0 + P, :], in_=hst[:])
                    xst = ev.tile([P, C_HID], F32, tag="xst")
                    nc.vector.tensor_copy(out=xst[:], in_=acc[:, C_HID:])
                    nc.sync.dma_start(out=xr_out[r0:r0 + P, :], in_=xst[:])
    nc.compile()
    return nc


def _build_l2(B, k_w):
    nc = bacc.Bacc("TRN2", target_bir_lowering=False, debug=False,
                   num_devices=NCORES)
    h_full = nc.dram_tensor("h_full", [NCORES * R, C_HID], BF16,
                            kind="ExternalInput")
    xr = nc.dram_tensor("xr", [R, C_HID], F32, kind="ExternalInput")
    st = nc.dram_tensor("st", [P, B * P], BF16, kind="ExternalInput")
    offs = nc.dram_tensor("offs", [P, B], I32, kind="ExternalInput")
    invd = nc.dram_tensor("invd", [P, NWIN], F32, kind="ExternalInput")
    w2 = nc.dram_tensor("w2", [C_HID, 2 * C_OUT], BF16, kind="ExternalInput")
    h2_out = nc.dram_tensor("h2_out", [R, C_OUT], BF16, kind="ExternalOutput")
    x2r_out = nc.dram_tensor("x2r_out", [R, C_OUT], F32, kind="ExternalOutput")

    from concourse.masks import make_identity
    bstart = np.concatenate(([0], np.cumsum(k_w)))
    with tile.TileContext(nc) as tc:
        with tc.tile_pool(name="cst", bufs=1) as cst, \
             tc.tile_pool(name="stp", bufs=3) as stp, \
             tc.tile_pool(name="gp", bufs=8) as gp, \
             tc.tile_pool(name="ev", bufs=3) as ev, \
             tc.tile_pool(name="ps", bufs=3, space="PSUM") as ps, \
             tc.tile_pool(name="pst", bufs=2, space="PSUM") as pst:
            offst = cst.tile([P, B], I32)
            nc.sync.dma_start(out=offst[:], in_=offs[:])
            invdt = cst.tile([P, NWIN], F32)
            nc.sync.dma_start(out=invdt[:], in_=invd[:])
            w2t = cst.tile([P, 2 * C_OUT], BF16)
            nc.sync.dma_start(out=w2t[:], in_=w2[:])
            ident = cst.tile([P, P], BF16)
            make_identity(nc, ident[:])

            for w in range(NWIN):
                b0, kw = int(bstart[w]), int(k_w[w])
                stt = stp.tile([P, kw * P], BF16, tag="stt")
                nc.sync.dma_start(out=stt[:], in_=st[:, b0 * P:(b0 + kw) * P])
                acc = ps.tile([P, C_HID], F32, space="PSUM", tag="acc")
                for j in range(kw):
                    gt = gp.tile([P, C_HID], BF16, tag="gt")
                    nc.gpsimd.indirect_dma_start(
                        out=gt[:], out_offset=None, in_=h_full[:],
                        in_offset=bass.IndirectOffsetOnAxis(
                            ap=offst[:, b0 + j:b0 + j + 1], axis=0))
                    nc.tensor.matmul(
                        out=acc[:], lhsT=stt[:, j * P:(j + 1) * P], rhs=gt[:],
                        start=(j == 0), stop=(j == kw - 1))
                # mean + self path + relu -> x2 (bf16)
                xrt = ev.tile([P, C_HID], F32, tag="xrt")
                nc.sync.dma_start(out=xrt[:], in_=xr[w * P:(w + 1) * P, :])
                mean = ev.tile([P, C_HID], F32, tag="mean")
                nc.vector.tensor_scalar_mul(mean[:], acc[:], invdt[:, w:w + 1])
                nc.vector.tensor_add(out=mean[:], in0=mean[:], in1=xrt[:])
                x2 = ev.tile([P, C_HID], BF16, tag="x2")
                nc.scalar.activation(x2[:], mean[:],
                                     mybir.ActivationFunctionType.Relu)
                # transpose x2 -> [chan, r] for the layer-2 projection
                x2tp = pst.tile([P, P], BF16, space="PSUM", tag="x2tp")
                nc.tensor.transpose(out=x2tp[:], in_=x2[:], identity=ident[:])
                x2t = ev.tile([P, P], BF16, tag="x2t")
                nc.vector.tensor_copy(out=x2t[:], in_=x2tp[:])
                acc2 = pst.tile([P, 2 * C_OUT], F32, space="PSUM", tag="acc2")
                nc.tensor.matmul(out=acc2[:], lhsT=x2t[:], rhs=w2t[:],
                                 start=True, stop=True)
                h2st = ev.tile([P, C_OUT], BF16, tag="h2st")
                nc.scalar.copy(out=h2st[:], in_=acc2[:, :C_OUT])
                nc.sync.dma_start(out=h2_out[w * P:(w + 1) * P, :], in_=h2st[:])
                x2st = ev.tile([P, C_OUT], F32, tag="x2st")
                nc.vector.tensor_copy(out=x2st[:], in_=acc2[:, C_OUT:])
                nc.sync.dma_start(out=x2r_out[w * P:(w + 1) * P, :], in_=x2st[:])
    nc.compile()
    return nc


def _build_l3(B, k_w):
    nc = bacc.Bacc("TRN2", target_bir_lowering=False, debug=False,
                   num_devices=NCORES)
    h2_full = nc.dram_tensor("h2_full", [NCORES * R, C_OUT], BF16,
                             kind="ExternalInput")
    x2r = nc.dram_tensor("x2r", [R, C_OUT], F32, kind="ExternalInput")
    st = nc.dram_tensor("st", [P, B * P], BF16, kind="ExternalInput")
    offs = nc.dram_tensor("offs", [P, B], I32, kind="ExternalInput")
    invd = nc.dram_tensor("invd", [P, NWIN], F32, kind="ExternalInput")
    b2r = nc.dram_tensor("b2r", [P, C_OUT], F32, kind="ExternalInput")
    out = nc.dram_tensor("out", [R, C_OUT], F32, kind="ExternalOutput")

    bstart = np.concatenate(([0], np.cumsum(k_w)))
    with tile.TileContext(nc) as tc:
        with tc.tile_pool(name="cst", bufs=1) as cst, \
             tc.tile_pool(name="stp", bufs=3) as stp, \
             tc.tile_pool(name="gp", bufs=8) as gp, \
             tc.tile_pool(name="ev", bufs=3) as ev, \
             tc.tile_pool(name="ps", bufs=3, space="PSUM") as ps:
            offst = cst.tile([P, B], I32)
            nc.sync.dma_start(out=offst[:], in_=offs[:])
            invdt = cst.tile([P, NWIN], F32)
            nc.sync.dma_start(out=invdt[:], in_=invd[:])
            b2t = cst.tile([P, C_OUT], F32)
            nc.sync.dma_start(out=b2t[:], in_=b2r[:])

            for w in range(NWIN):
                b0, kw = int(bstart[w]), int(k_w[w])
                stt = stp.tile([P, kw * P], BF16, tag="stt")
                nc.sync.dma_start(out=stt[:], in_=st[:, b0 * P:(b0 + kw) * P])
                acc = ps.tile([P, C_OUT], F32, space="PSUM", tag="acc")
                for j in range(kw):
                    gt = gp.tile([P, C_OUT], BF16, tag="gt")
                    nc.gpsimd.indirect_dma_start(
                        out=gt[:], out_offset=None, in_=h2_full[:],
                        in_offset=bass.IndirectOffsetOnAxis(
                            ap=offst[:, b0 + j:b0 + j + 1], axis=0))
                    nc.tensor.matmul(
                        out=acc[:], lhsT=stt[:, j * P:(j + 1) * P], rhs=gt[:],
                        start=(j == 0), stop=(j == kw - 1))
                x2rt = ev.tile([P, C_OUT], F32, tag="x2rt")
                nc.sync.dma_start(out=x2rt[:], in_=x2r[w * P:(w + 1) * P, :])
                mean = ev.tile([P, C_OUT], F32, tag="mean")
                nc.vector.tensor_scalar_mul(mean[:], acc[:], invdt[:, w:w + 1])
                nc.vector.tensor_add(out=mean[:], in0=mean[:], in1=x2rt[:])
                nc.vector.tensor_add(out=mean[:], in0=mean[:], in1=b2t[:])
                nc.sync.dma_start(out=out[w * P:(w + 1) * P, :], in_=mean[:])
    nc.compile()
    return nc


# ------------------------------------------------------------------- driver
def _run(nc, in_maps, trace=False):
    res = bass_utils.run_bass_kernel_spmd(
        nc, in_maps, core_ids=list(range(NCORES)), trace=trace)
    if res.exec_time_ns:
        _EXEC_NS.append(res.exec_time_ns)
    return res.results


def kernel(features, edges, edges2, edge_features,
           W1_l, b1_l, W1_r, W2_l, b2_l, W2_r, _trace=False):
    features = np.asarray(features, np.float32)
    src = np.asarray(edges[0], np.int64)
    dst = np.asarray(edges[1], np.int64)
    _EXEC_NS.clear()

    # ---- host prep
    B, offs_all, st_all, invd_all = _prep_edges(src, dst)
    # recover k_w from block starts: infer from prep (recompute cheaply)
    deg = np.bincount(dst, minlength=N_NODES)
    core = dst // SHARD
    win = (dst - core * SHARD) // P
    counts = np.zeros((NCORES, NWIN), np.int64)
    for m in range(NCORES):
        counts[m] = np.bincount(win[core == m], minlength=NWIN)
    k_w = np.maximum(1, (counts.max(axis=0) + P - 1) // P)
    assert int(k_w.sum()) == B

    w1aug = np.zeros((CIN_PAD, 2 * C_HID), np.float32)
    w1aug[:C_IN, :C_HID] = np.asarray(W1_l, np.float32)
    w1aug[:C_IN, C_HID:] = np.asarray(W1_r, np.float32)
    w1aug[C_IN, C_HID:] = np.asarray(b1_l, np.float32)   # bias via 1-row
    w1aug = _bf16(w1aug)

    w2c = _bf16(np.concatenate([np.asarray(W2_l, np.float32),
                                np.asarray(W2_r, np.float32)], axis=1))
    b2rep = np.ascontiguousarray(
        np.broadcast_to(np.asarray(b2_l, np.float32), (P, C_OUT)))

    xts = []
    for m in range(NCORES):
        xt = np.zeros((CIN_PAD, R), np.float32)
        xt[:C_IN, :SHARD] = features[m * SHARD:(m + 1) * SHARD].T
        xt[C_IN, :SHARD] = 1.0
        xts.append(np.ascontiguousarray(xt))

    # ---- L1: projections
    nc1 = _build_l1()
    res1 = _run(nc1, [dict(xt=xts[m], w1=w1aug) for m in range(NCORES)],
                trace=_trace)
    h_full = np.concatenate([res1[m]["h_out"] for m in range(NCORES)], axis=0)
    h_full = np.ascontiguousarray(h_full)

    # ---- L2: layer-1 aggregation + layer-2 projections
    nc2 = _build_l2(B, k_w)
    res2 = _run(nc2, [dict(h_full=h_full, xr=res1[m]["xr_out"],
                           st=st_all[m], offs=offs_all[m],
                           invd=invd_all[m], w2=w2c)
                      for m in range(NCORES)], trace=_trace)
    h2_full = np.concatenate([res2[m]["h2_out"] for m in range(NCORES)],
                             axis=0)
    h2_full = np.ascontiguousarray(h2_full)

    # ---- L3: layer-2 aggregation + output
    nc3 = _build_l3(B, k_w)
    res3 = _run(nc3, [dict(h2_full=h2_full, x2r=res2[m]["x2r_out"],
                           st=st_all[m], offs=offs_all[m],
                           invd=invd_all[m], b2r=b2rep)
                      for m in range(NCORES)], trace=_trace)

    out = np.concatenate([res3[m]["out"][:SHARD] for m in range(NCORES)],
                         axis=0)
    return np.ascontiguousarray(out, dtype=np.float32)
